# revision 24
# baseline (speedup 1.0000x reference)
"""Trainium2 Bass kernel for EnhancedSpikingAudioNet (4-layer LIF SNN).

Network (eval mode): for t in 0..99:
    s1,m1 = LIF(x_t @ W1.T + b1, m1)
    s2,m2 = LIF(s1 @ W2.T + b2, m2)
    s3,m3 = LIF(s2 @ W3.T + b3, m3)
    s4,m4 = LIF(s3 @ W4.T + b4, m4)
returns m4 (final step), shape [B=256, 10].

LIF (snnTorch Leaky, reset_mechanism='subtract', beta=.95, thr=1):
    reset = (m_prev > 1);  m = beta*m_prev + cur - reset;  s = (m > 1)

Strategy: data-parallel over batch (32 per core, 8 cores).  Inside a
core, time is blocked (TB=10): all matmuls for a block are batched over
the block's 10 steps (moving free dim N=320); only the per-step LIF
update is sequential.  Layout: features on partitions (128-chunks),
(t, batch) on the free dim.  PSUM drains to SBUF via ScalarE with the
layer bias fused in.

LIF chain (bit-identical sign-flipped form): track mm = -m.  Per step,
two same-engine DVE ops (shortest possible serial chain):
    tmp   = (mm * beta) - cur            # == -(beta*m + cur)
    mm_t  = (mm is_lt -1) + tmp          # == -((beta*m+cur) - (m>1))
Every fp32 op is the exact sign-mirror of the upstream form (RNE is
sign-symmetric), so results match the previous 3-op chain bit-for-bit;
the host negates the final mm4.  Spikes for the next layer's matmul,
s = (mm is_lt -1), are generated per step on GPSIMD off the critical
path (layer 4's spikes feed nothing and are skipped).

Numerics: the spike cascade amplifies matmul noise (a plain f32r
matmul gives ~16% output error; even exact-fp32 summation-order noise
gives ~1.6%; gate is 2%), so matmuls must be fp32-faithful.  Hardware
probing established: float32r = operands rounded RNE to 11 mantissa
bits, then EXACT products with clean fp32 accumulation, at full PE
rate (1 cyc/row) for moving dims >= 256.  Every fp32 tensor is split
host-side into two 11-bit planes (h = rne11(a), l = a-h; both planes
survive the hw operand rounding unchanged).  Layer 1 accumulates
wh@xh + wh@xl + wl@xh (dropping wl@xl ~ 2^-24), layers 2-4 accumulate
wh@s + wl@s.  Accumulation order per PSUM bank (k ascending, fixed
term order) is kept identical across scheduling changes.

Schedule: weights are loaded per-128-chunk and layer 1 runs its k loop
OUTERMOST (6 PSUM banks open at once), so the first matmul needs only
x(blk0) chunk k0 + W1 chunk k0 instead of all of W1; DMA issue order
(x blk0 chunks, W1 chunks h/l-interleaved, biases, x blk1 chunks,
W2..W4) keeps the PE fed from ~6us on a FIFO DMA engine.
"""

import os
import sys

import numpy as np

for _p in ("/opt/trn_rl_repo",):
    if os.path.isdir(_p) and _p not in sys.path:
        sys.path.insert(0, _p)

import concourse.bass as bass
import concourse.mybir as mybir
import concourse.tile as tile
from concourse import bass_utils

F32 = mybir.dt.float32
F32R = mybir.dt.float32r
ALU = mybir.AluOpType
ACTF = mybir.ActivationFunctionType
PLANES = ("h", "l")  # 11-bit f32r planes


def _patch_tail_drain():
    """This container's walrus allows only ONE sync-wait on a Drain
    instruction; Tile's kernel-tail drain can carry several (one per DMA
    HW queue).  Spread the waits across consecutive drains instead."""
    from concourse.vector_clock import ScopedClock

    if getattr(tile.TileContext, "_tail_drain_patched", False):
        return

    def _drain_and_barrier(self, tick_clock, wait_clock):
        drain_inst = self.nc.sync.drain()
        wait_clock.add_sem_waits(
            drain_inst.ins, ScopedClock({None: tick_clock.global_clock})
        )
        si = drain_inst.ins.sync_info
        if si is not None and si.on_wait and len(si.on_wait) > 1:
            waits = list(si.on_wait)
            drain_inst.ins.sync_info = mybir.SyncInfo(
                on_wait=[waits[0]], on_update=list(si.on_update or [])
            )
            for w in waits[1:]:
                extra = self.nc.sync.drain()
                extra.ins.sync_info = mybir.SyncInfo(on_wait=[w], on_update=[])

        self.nc.all_engine_barrier()
        assert self.sems is not None
        popped = self.nc._tile_sem_poison_stack.pop()
        assert popped is self._sem_poison
        self.nc.clear_and_free_semaphores(
            list(self.sems.allocated().values())
        )
        self.nc.all_engine_barrier()

    tile.TileContext._drain_and_barrier = _drain_and_barrier
    tile.TileContext._tail_drain_patched = True


_patch_tail_drain()


def _split_multi_waits(nc):
    """This walrus build rejects instructions carrying more than one
    sync-wait (a DMA-HW-queue sem wait expands into several wait
    commands).  Give every instruction at most one wait; extras go onto
    same-engine NOPs inserted immediately before it."""

    def fresh_nop(engine):
        eng = nc.engines[engine]
        bi = eng.nop(nofuse=True)
        raw = bi.ins
        # nop() appended raw to the current bb -- remove it, we re-insert.
        for bb in nc.main_func.blocks:
            try:
                bb.instructions.remove(raw)
                break
            except ValueError:
                continue
        return raw

    for bb in nc.main_func.blocks:
        insts = bb.instructions
        i = 0
        while i < len(insts):
            ins = insts[i]
            si = getattr(ins, "sync_info", None)
            ow = list(si.on_wait) if (si is not None and si.on_wait) else []
            if len(ow) > 1:
                upd = list(si.on_update or [])
                for w in ow[:-1]:
                    nop = fresh_nop(ins.engine)
                    nop.sync_info = mybir.SyncInfo(on_wait=[w], on_update=[])
                    insts.insert(i, nop)
                    i += 1
                ins.sync_info = mybir.SyncInfo(on_wait=[ow[-1]],
                                               on_update=upd)
            i += 1


T, B, D = 100, 256, 1024
HH = [1024, 768, 512, 256, 10]  # H[l-1] -> H[l] for layer l in 1..4
NCORES = 8
BC = B // NCORES  # 32 batch per core
TB = 10           # time block
NBLK = T // TB
RING = 2 * TB     # ring slots for cur/spike buffers
NMR = 8           # membrane-ring slots (allows 4-step batched spike ops)
SG = 4            # spike-op step grouping (aligned to absolute step)
BETA = 0.95


def _kch(l):  # contraction chunks for layer l (input feature chunks)
    return (HH[l - 1] + 127) // 128


def _mch(l):  # output feature chunks
    return (HH[l] + 127) // 128


def _mpart(l):  # partitions used by last output chunk
    r = HH[l] % 128
    return 128 if r == 0 else r


def build_nc(repeat=1):
    nc = bass.Bass(target_bir_lowering=False, trn_type="TRN2")

    x_d = {
        p: nc.dram_tensor(f"x_{p}", [D, T * BC], F32R,
                          kind="ExternalInput") for p in PLANES
    }
    w_d = {}
    b_d = {}
    for l in range(1, 5):
        for p in PLANES:
            w_d[l, p] = nc.dram_tensor(
                f"w{l}{p}", [_kch(l), 128, HH[l]], F32R,
                kind="ExternalInput"
            )
        b_d[l] = nc.dram_tensor(f"b{l}", [HH[l]], F32, kind="ExternalInput")
    out_d = nc.dram_tensor("out", [10, BC], F32, kind="ExternalOutput")

    NB = TB * BC
    NT = NBLK * repeat  # total blocks emitted

    with tile.TileContext(nc) as tc:
        from contextlib import ExitStack

        with ExitStack() as ctx:
            wpool = ctx.enter_context(tc.tile_pool(name="weights", bufs=1))
            xpool = ctx.enter_context(tc.tile_pool(name="xblk", bufs=2))
            spool = ctx.enter_context(tc.tile_pool(name="state", bufs=1))
            psum = ctx.enter_context(
                tc.tile_pool(name="psum", bufs=7, space="PSUM")
            )
            pdum = ctx.enter_context(
                tc.tile_pool(name="pdum", bufs=1, space="PSUM")
            )

            # ---- per-chunk x DMA (separate tiles => per-chunk deps) ----
            def dma_x(blk):
                tiles = {}
                src = blk % NBLK
                for k in range(_kch(1)):
                    for p in PLANES:
                        xt = xpool.tile([128, NB], F32R,
                                        name=f"xb{p}{k}", tag=f"xb{p}{k}")
                        nc.sync.dma_start(
                            xt,
                            x_d[p][k * 128:(k + 1) * 128,
                                   src * NB:(src + 1) * NB],
                        )
                        tiles[p, k] = xt
                return tiles

            # ---- persistent state (allocate first: fixed SBUF homes) ----
            m_t = {}    # membrane rings, k-major: [pp, mc * NMR * BC]
            tmp_t = {}
            s_t = {}    # spike rings, k-major: [pp, mc * RING * BC]
            c_t = {}    # cur rings, t-major: [pp, RING * Fl]
            for l in range(1, 5):
                mc = _mch(l)
                mp = _mpart(l)
                Fl = mc * BC
                pp = mp if mc == 1 else 128
                m_t[l] = spool.tile([pp, mc * NMR * BC], F32, name=f"mem{l}")
                tmp_t[l] = spool.tile([pp, Fl], F32, name=f"tmp{l}")
                c_t[l] = spool.tile([pp, RING * Fl], F32, name=f"cur{l}")
                nc.vector.memset(m_t[l], 0.0)
                if l < 4:  # layer-4 spikes feed nothing
                    s_t[l] = spool.tile([pp, mc * RING * BC], F32R,
                                        name=f"spk{l}")
                    nc.vector.memset(s_t[l].bitcast(F32), 0.0)

            # ---- weights + biases: per-128-chunk tiles ----
            # DMA issue order sets the FIFO order on the DMA engine (and
            # the serial ~625ns/DMA HWDGE descriptor-gen): x(blk0) and W1
            # interleaved per chunk so the first matmul waits only on
            # chunk k0, then biases, x(blk1), then W2..W4 (first needed
            # one tick later).
            w_sb = {}   # (l, plane, k) -> [128, HH[l]]
            b_sb = {}
            x_tiles = {0: {}}
            for k in range(_kch(1)):
                for p in PLANES:
                    xt = xpool.tile([128, NB], F32R,
                                    name=f"xb{p}{k}", tag=f"xb{p}{k}")
                    nc.sync.dma_start(
                        xt, x_d[p][k * 128:(k + 1) * 128, 0:NB]
                    )
                    x_tiles[0][p, k] = xt
                for p in PLANES:
                    w_sb[1, p, k] = wpool.tile([128, HH[1]], F32R,
                                               name=f"wsb1{p}{k}")
                    nc.sync.dma_start(w_sb[1, p, k], w_d[1, p][k])

            for l in range(1, 5):
                mp = _mpart(l)
                b_sb[l] = wpool.tile([128, _mch(l)], F32, name=f"bsb{l}")
                nc.sync.dma_start(
                    b_sb[l][:mp, :],
                    b_d[l].rearrange("(c q) -> q c", q=mp)
                    if _mch(l) > 1
                    else b_d[l][:].unsqueeze(-1),
                )

            x_tiles[1] = dma_x(1)

            def dma_w(l):
                for k in range(_kch(l)):
                    for p in PLANES:
                        w_sb[l, p, k] = wpool.tile([128, HH[l]], F32R,
                                                   name=f"wsb{l}{p}{k}")
                        nc.sync.dma_start(w_sb[l, p, k], w_d[l, p][k])

            # W2 is needed one tick in; W3/W4 are deferred behind the
            # x(blk2) prefetch so they don't delay it in the DMA FIFO.
            dma_w(2)

            def lif_steps(l, b, split=False):
                """Sequential LIF updates for layer l over global block b.

                Two DVE ops per step (see module docstring); spike
                materialization on GPSIMD off the chain (skipped for l=4).

                split=True (used for the final block, where the chain is
                the critical path): run the recurrence as independent
                sub-chains over m-chunk pairs.  Each sub-chain only waits
                for its own chunks' cur drains, so it overlaps the tail
                of the same layer's matmul phase.  The recurrence is
                elementwise per neuron, so values are bit-identical.
                """
                mc = _mch(l)
                sb = (b % 2) * TB
                mr = m_t[l].rearrange("q (k n b) -> q k n b", n=NMR, b=BC)
                tmp3 = tmp_t[l].rearrange("q (k b) -> q k b", b=BC)
                c4 = c_t[l].rearrange("q (r k b) -> q r k b", r=RING, b=BC)
                if l < 4:
                    s4 = s_t[l].rearrange("q (k r b) -> q k r b",
                                          r=RING, b=BC)
                if not split:
                    # lanes: one full-width chain
                    pairs = [[(0, mc, 0, BC)]]
                elif mc >= 2:
                    # exactly two chunk-half lanes, ops interleaved: each
                    # lane's ~95ns dependent-op pipeline lag is covered by
                    # the other, at minimal extra per-op fixed cost
                    h = (mc + 1) // 2
                    pairs = [[(0, h, 0, BC), (h, mc, 0, BC)]]
                else:
                    # single chunk: interleave two batch-half lanes
                    h = BC // 2
                    pairs = [[(0, 1, 0, h), (0, 1, h, BC)]]
                for lanes in pairs:
                    t0 = 0  # start of the current spike group
                    for t in range(TB):
                        g = b * TB + t
                        cu, pv = g % NMR, (g - 1) % NMR
                        for k0, k1, b0, b1 in lanes:
                            # tmp = (mm * beta) - cur
                            nc.vector.scalar_tensor_tensor(
                                tmp3[:, k0:k1, b0:b1],
                                mr[:, k0:k1, pv, b0:b1], BETA,
                                c4[:, sb + t, k0:k1, b0:b1],
                                op0=ALU.mult, op1=ALU.subtract,
                            )
                        for k0, k1, b0, b1 in lanes:
                            # mm = (mm_prev is_lt -1) + tmp
                            nc.vector.scalar_tensor_tensor(
                                mr[:, k0:k1, cu, b0:b1],
                                mr[:, k0:k1, pv, b0:b1], -1.0,
                                tmp3[:, k0:k1, b0:b1],
                                op0=ALU.is_lt, op1=ALU.add,
                            )
                        # batched spikes: s[t0..t] = mm_ring < -1, flushed
                        # on SG-aligned absolute-step boundaries so ring
                        # slots stay contiguous (never wrap mod NMR).
                        # Tail chains flush every 2 steps so the next
                        # layer's matmuls unblock sooner.
                        sg = 2 if split else SG
                        if l < 4 and (g % sg == sg - 1 or t == TB - 1):
                            s0 = (b * TB + t0) % NMR
                            ng = t - t0 + 1
                            for k0, k1, b0, b1 in lanes:
                                nc.gpsimd.tensor_scalar(
                                    s4[:, k0:k1, sb + t0:sb + t + 1, b0:b1],
                                    mr[:, k0:k1, s0:s0 + ng, b0:b1], -1.0,
                                    None, op0=ALU.is_lt,
                                )
                            t0 = t + 1

            def layer_matmul(l, b, terms_of_k, k_outer=False,
                             splits=None):
                """Batched matmuls for layer l over global block b.

                terms_of_k(k) -> list of (plane, rhs AP [128, TB*BC]) to
                accumulate.  Per-PSUM accumulation order is k ascending
                with terms_of_k's order within k, identical for every
                loop nesting and step split (per-element accumulation
                order never changes).  Drains psum to c_t[l] with bias
                fused.

                splits: list of (t0, t1) step sub-ranges, each its own
                PSUM group + drain; used for the final block's layer-4
                matmul so its LIF chain starts before the last spike
                groups land.
                """
                mc = _mch(l)
                kc = _kch(l)
                mp = _mpart(l)
                sb = (b % 2) * TB
                c4 = c_t[l].rearrange("q (r k b) -> q r k b", r=RING, b=BC)
                nterms = len(terms_of_k(0))
                ntot = kc * nterms
                if splits is None:
                    splits = [(0, TB)]

                def emit(ps, m, pp, k, wp, rhs, i, t0, t1):
                    lhsT = w_sb[l, wp, k][:, m * 128:m * 128 + pp]
                    nc.tensor.matmul(
                        ps, lhsT, rhs[:, t0 * BC:t1 * BC],
                        start=(i == 0), stop=(i == ntot - 1),
                    )

                def drain(ps, m, pp, t0, t1):
                    nc.scalar.activation(
                        c4[:pp, sb + t0:sb + t1, m, :],
                        ps.rearrange("q (t b) -> q t b", b=BC),
                        ACTF.Identity,
                        bias=b_sb[l][:pp, m:m + 1],
                    )

                if k_outer:
                    t0, t1 = splits[0]
                    tiles = []
                    for m in range(mc):
                        pp = mp if m == mc - 1 else 128
                        tiles.append(
                            psum.tile([pp, (t1 - t0) * BC], F32,
                                      name=f"ps{l}", tag="ps")
                        )
                    for k in range(kc):
                        terms = terms_of_k(k)
                        for m in range(mc):
                            pp = mp if m == mc - 1 else 128
                            for j, (wp, rhs) in enumerate(terms):
                                emit(tiles[m], m, pp, k, wp, rhs,
                                     k * nterms + j, t0, t1)
                    for m in range(mc):
                        pp = mp if m == mc - 1 else 128
                        drain(tiles[m], m, pp, t0, t1)
                else:
                    for m in range(mc):
                        pp = mp if m == mc - 1 else 128
                        for t0, t1 in splits:
                            ps = psum.tile([pp, (t1 - t0) * BC], F32,
                                           name=f"ps{l}", tag="ps")
                            for k in range(kc):
                                for j, (wp, rhs) in \
                                        enumerate(terms_of_k(k)):
                                    emit(ps, m, pp, k, wp, rhs,
                                         k * nterms + j, t0, t1)
                            drain(ps, m, pp, t0, t1)

            def spike_terms(l, b):
                sb = (b % 2) * TB
                sl = s_t[l - 1]

                def terms(k, sl=sl, sb=sb):
                    rhs = sl[:, k * RING * BC + sb * BC:
                             k * RING * BC + sb * BC + NB]
                    return [(p, rhs) for p in PLANES]

                return terms

            def warm_pe(l, b, n):
                """Dummy matmuls gated on the tail chain's early spike
                groups: each executes as its group lands, keeping the PE
                p-state ramp alive across the chain-wait gap before the
                next layer's real matmuls (idle >~3us drops the clock)."""
                sb = (b % 2) * TB
                sl = s_t[l]
                ps = pdum.tile([128, 64], F32, name="psd", tag="psd")
                for i in range(n):
                    rhs = sl[:, sb * BC + i * 2 * BC:
                             sb * BC + (i * 2 + 2) * BC]
                    nc.tensor.matmul(ps, w_sb[1, "h", 0][:, :128],
                                     rhs, start=True, stop=True)

            # Software pipeline: at tick t, layer l works on block t-(l-1);
            # the PE's matmuls for tick t depend only on LIF work emitted
            # at tick t-1, so the PE never waits on the DVE in steady
            # state.  Repeats (timing runs) just extend the tick range.
            nticks = NT + 4
            for tick in range(nticks):
                # prefetch next tick's x block (this tick's is resident)
                if 2 <= tick + 1 < NT:
                    x_tiles[tick + 1] = dma_x(tick + 1)
                if tick == 1:
                    dma_w(3)
                    dma_w(4)
                for l in (1, 2, 3, 4):
                    b = tick - (l - 1)
                    if not (0 <= b < NT):
                        continue
                    if l == 1:
                        xb = x_tiles[b]
                        layer_matmul(
                            1, b,
                            lambda k: [
                                ("h", xb["h", k]),
                                ("h", xb["l", k]),
                                ("l", xb["h", k]),
                            ],
                            # k-outer only while DMA-paced (weights still
                            # streaming in); m-outer afterwards so cur
                            # drains land incrementally for the LIF chain
                            k_outer=(b <= 1),
                        )
                    else:
                        layer_matmul(
                            l, b, spike_terms(l, b),
                            # final-block L4: two step sub-ranges so its
                            # chain starts before the last s3 spikes land
                            # (4 tiles -> the <256-row penalty is ~0.3us)
                            splits=([(0, 8), (8, TB)]
                                    if l == 4 and b == NT - 1 else None),
                        )
                    if b - 1 >= 0:
                        # late-pipeline chains run in the DVE-latency-
                        # dominated tail window where no other chain fills
                        # their dependent-op bubbles: 2-lane them too
                        late = b - 1 >= NT - 2 or (l == 4 and
                                                   b - 1 >= NT - 3)
                        lif_steps(l, b - 1, split=late)
                if tick >= NT - 1:
                    # drain the tail of each layer's LIF chain; split
                    # sub-chains overlap the final matmul phases.  The
                    # warm-up dummies (emitted after the chain, so they
                    # wait on its early spike groups) bridge the PE-idle
                    # window until the next layer's real matmuls.
                    for l in (1, 2, 3, 4):
                        if tick - (l - 1) == NT - 1:
                            lif_steps(l, tick - l + 1,
                                      split=True)
                            if l in (2, 3):
                                warm_pe(l, NT - 1, 3)

            # final-step membrane of layer 4 (sign-flipped; host negates)
            gfin = NT * TB - 1
            mr4 = m_t[4].rearrange("q (k n b) -> q k n b", n=NMR, b=BC)
            nc.sync.dma_start(out_d[:, :], mr4[:, 0, gfin % NMR, :])

    _split_multi_waits(nc)
    return nc


_NC_CACHE = None


def _get_nc():
    global _NC_CACHE
    if _NC_CACHE is None:
        _NC_CACHE = build_nc()
    return _NC_CACHE


def _rne11(a):
    """Round fp32 mantissa to 11 bits (RNE) -- the f32r operand grid."""
    u = np.ascontiguousarray(a, np.float32).view(np.uint32).astype(np.uint64)
    zb = 12  # 23 - 11
    lsb = (u >> zb) & 1
    add = lsb + ((1 << (zb - 1)) - 1)
    r = ((u + add) >> zb) << zb
    return r.astype(np.uint32).view(np.float32)


def _split2_11(a):
    """fp32 -> two 11-bit-mantissa planes with h + l == a exactly."""
    a = np.asarray(a, np.float32)
    h = _rne11(a)
    l = (a - h).astype(np.float32)
    return h, l


def prep_inputs(x, W1, b1, W2, b2, W3, b3, W4, b4):
    """Full inputs -> per-core in_maps."""
    Ws = {1: W1, 2: W2, 3: W3, 4: W4}
    bs = {1: b1, 2: b2, 3: b3, 4: b4}
    shared = {}
    for l in range(1, 5):
        wt = np.ascontiguousarray(
            np.asarray(Ws[l], np.float32).T.reshape(_kch(l), 128, HH[l])
        )
        wh, wl = _split2_11(wt)
        shared[f"w{l}h"] = wh
        shared[f"w{l}l"] = wl
        shared[f"b{l}"] = np.ascontiguousarray(bs[l], dtype=np.float32)
    in_maps = []
    for c in range(NCORES):
        xc = np.asarray(x[:, c * BC:(c + 1) * BC, :], np.float32)
        xc = np.ascontiguousarray(xc.transpose(2, 0, 1).reshape(D, T * BC))
        xh, xl = _split2_11(xc)
        m = {"x_h": xh, "x_l": xl}
        m.update(shared)
        in_maps.append(m)
    return in_maps


def run(in_maps, trace=False):
    nc = _get_nc()
    return bass_utils.run_bass_kernel_spmd(
        nc, in_maps, core_ids=list(range(NCORES)), trace=trace
    )


def kernel(**inputs):
    in_maps = prep_inputs(**inputs)
    res = run(in_maps)
    out = np.empty((B, 10), dtype=np.float32)
    for c in range(NCORES):
        # device tracks -m (sign-flipped LIF chain); negate here
        out[c * BC:(c + 1) * BC, :] = -res.results[c]["out"].T
    return out


def bench(in_maps, iters=20, nc=None):
    """Repeat-execute the kernel via a cached sharded jit; returns list of
    per-call wall times (seconds).  Mirrors bass2jax.run_bass_via_pjrt's
    multi-core path but keeps inputs device-resident across calls."""
    import time

    import jax
    import concourse.mybir as mybir_
    from jax.sharding import Mesh, PartitionSpec
    from jax.experimental.shard_map import shard_map
    from concourse import bass2jax

    bass2jax.install_neuronx_cc_hook()
    if nc is None:
        nc = _get_nc()

    part_name = (nc.partition_id_tensor.name
                 if nc.partition_id_tensor else None)
    in_names, out_names, out_avals, zero_outs = [], [], [], []
    for alloc in nc.m.functions[0].allocations:
        if not isinstance(alloc, mybir_.MemoryLocationSet):
            continue
        name = alloc.memorylocations[0].name
        if alloc.kind == "ExternalInput":
            if name != part_name:
                in_names.append(name)
        elif alloc.kind == "ExternalOutput":
            out_names.append(name)
            shape = tuple(alloc.tensor_shape)
            dtype = mybir_.dt.np(alloc.dtype)
            out_avals.append(jax.core.ShapedArray(shape, dtype))
            zero_outs.append(np.zeros(shape, dtype))
    n_params = len(in_names)
    all_in_names = in_names + out_names
    if part_name is not None:
        all_in_names = all_in_names + [part_name]

    def _body(*args):
        operands = list(args)
        if part_name is not None:
            operands.append(bass2jax.partition_id_tensor())
        outs = bass2jax._bass_exec_p.bind(
            *operands,
            out_avals=tuple(out_avals),
            in_names=tuple(all_in_names),
            out_names=tuple(out_names),
            lowering_input_output_aliases=(),
            sim_require_finite=True,
            sim_require_nnan=True,
            nc=nc,
        )
        return tuple(outs)

    devices = jax.devices()[:NCORES]
    mesh = Mesh(np.asarray(devices), ("core",))
    n_outs = len(out_names)
    sharded = jax.jit(
        shard_map(
            _body, mesh=mesh,
            in_specs=(PartitionSpec("core"),) * (n_params + n_outs),
            out_specs=(PartitionSpec("core"),) * n_outs,
            check_rep=False,
        ),
        donate_argnums=tuple(range(n_params, n_params + n_outs)),
        keep_unused=True,
    )
    concat_in = [
        np.concatenate([np.asarray(m[nm]) for m in in_maps], axis=0)
        for nm in in_names
    ]
    concat_in = jax.device_put(concat_in)
    zeros = [
        np.zeros((NCORES * z.shape[0], *z.shape[1:]), z.dtype)
        for z in zero_outs
    ]
    # warmup (compile)
    out = sharded(*concat_in, *zeros)
    jax.block_until_ready(out)
    times = []
    for _ in range(iters):
        t0 = time.perf_counter()
        out = sharded(*concat_in, *zeros)
        jax.block_until_ready(out)
        times.append(time.perf_counter() - t0)
    return times


# revision 25
# speedup vs baseline: 1.0066x; 1.0066x over previous
"""Trainium2 Bass kernel for EnhancedSpikingAudioNet (4-layer LIF SNN).

Network (eval mode): for t in 0..99:
    s1,m1 = LIF(x_t @ W1.T + b1, m1)
    s2,m2 = LIF(s1 @ W2.T + b2, m2)
    s3,m3 = LIF(s2 @ W3.T + b3, m3)
    s4,m4 = LIF(s3 @ W4.T + b4, m4)
returns m4 (final step), shape [B=256, 10].

LIF (snnTorch Leaky, reset_mechanism='subtract', beta=.95, thr=1):
    reset = (m_prev > 1);  m = beta*m_prev + cur - reset;  s = (m > 1)

Strategy: data-parallel over batch (32 per core, 8 cores).  Inside a
core, time is blocked (TB=10): all matmuls for a block are batched over
the block's 10 steps (moving free dim N=320); only the per-step LIF
update is sequential.  Layout: features on partitions (128-chunks),
(t, batch) on the free dim.  PSUM drains to SBUF via ScalarE with the
layer bias fused in.

LIF chain (bit-identical sign-flipped form): track mm = -m.  Per step,
two same-engine DVE ops (shortest possible serial chain):
    tmp   = (mm * beta) - cur            # == -(beta*m + cur)
    mm_t  = (mm is_lt -1) + tmp          # == -((beta*m+cur) - (m>1))
Every fp32 op is the exact sign-mirror of the upstream form (RNE is
sign-symmetric), so results match the previous 3-op chain bit-for-bit;
the host negates the final mm4.  Spikes for the next layer's matmul,
s = (mm is_lt -1), are generated per step on GPSIMD off the critical
path (layer 4's spikes feed nothing and are skipped).

Numerics: the spike cascade amplifies matmul noise (a plain f32r
matmul gives ~16% output error; even exact-fp32 summation-order noise
gives ~1.6%; gate is 2%), so matmuls must be fp32-faithful.  Hardware
probing established: float32r = operands rounded RNE to 11 mantissa
bits, then EXACT products with clean fp32 accumulation, at full PE
rate (1 cyc/row) for moving dims >= 256.  Every fp32 tensor is split
host-side into two 11-bit planes (h = rne11(a), l = a-h; both planes
survive the hw operand rounding unchanged).  Layer 1 accumulates
wh@xh + wh@xl + wl@xh (dropping wl@xl ~ 2^-24), layers 2-4 accumulate
wh@s + wl@s.  Accumulation order per PSUM bank (k ascending, fixed
term order) is kept identical across scheduling changes.

Schedule: weights are loaded per-128-chunk and layer 1 runs its k loop
OUTERMOST (6 PSUM banks open at once), so the first matmul needs only
x(blk0) chunk k0 + W1 chunk k0 instead of all of W1; DMA issue order
(x blk0 chunks, W1 chunks h/l-interleaved, biases, x blk1 chunks,
W2..W4) keeps the PE fed from ~6us on a FIFO DMA engine.
"""

import os
import sys

import numpy as np

for _p in ("/opt/trn_rl_repo",):
    if os.path.isdir(_p) and _p not in sys.path:
        sys.path.insert(0, _p)

import concourse.bass as bass
import concourse.mybir as mybir
import concourse.tile as tile
from concourse import bass_utils

F32 = mybir.dt.float32
F32R = mybir.dt.float32r
ALU = mybir.AluOpType
ACTF = mybir.ActivationFunctionType
PLANES = ("h", "l")  # 11-bit f32r planes


def _patch_tail_drain():
    """This container's walrus allows only ONE sync-wait on a Drain
    instruction; Tile's kernel-tail drain can carry several (one per DMA
    HW queue).  Spread the waits across consecutive drains instead."""
    from concourse.vector_clock import ScopedClock

    if getattr(tile.TileContext, "_tail_drain_patched", False):
        return

    def _drain_and_barrier(self, tick_clock, wait_clock):
        drain_inst = self.nc.sync.drain()
        wait_clock.add_sem_waits(
            drain_inst.ins, ScopedClock({None: tick_clock.global_clock})
        )
        si = drain_inst.ins.sync_info
        if si is not None and si.on_wait and len(si.on_wait) > 1:
            waits = list(si.on_wait)
            drain_inst.ins.sync_info = mybir.SyncInfo(
                on_wait=[waits[0]], on_update=list(si.on_update or [])
            )
            for w in waits[1:]:
                extra = self.nc.sync.drain()
                extra.ins.sync_info = mybir.SyncInfo(on_wait=[w], on_update=[])

        self.nc.all_engine_barrier()
        assert self.sems is not None
        popped = self.nc._tile_sem_poison_stack.pop()
        assert popped is self._sem_poison
        self.nc.clear_and_free_semaphores(
            list(self.sems.allocated().values())
        )
        self.nc.all_engine_barrier()

    tile.TileContext._drain_and_barrier = _drain_and_barrier
    tile.TileContext._tail_drain_patched = True


_patch_tail_drain()


def _split_multi_waits(nc):
    """This walrus build rejects instructions carrying more than one
    sync-wait (a DMA-HW-queue sem wait expands into several wait
    commands).  Give every instruction at most one wait; extras go onto
    same-engine NOPs inserted immediately before it."""

    def fresh_nop(engine):
        eng = nc.engines[engine]
        bi = eng.nop(nofuse=True)
        raw = bi.ins
        # nop() appended raw to the current bb -- remove it, we re-insert.
        for bb in nc.main_func.blocks:
            try:
                bb.instructions.remove(raw)
                break
            except ValueError:
                continue
        return raw

    for bb in nc.main_func.blocks:
        insts = bb.instructions
        i = 0
        while i < len(insts):
            ins = insts[i]
            si = getattr(ins, "sync_info", None)
            ow = list(si.on_wait) if (si is not None and si.on_wait) else []
            if len(ow) > 1:
                upd = list(si.on_update or [])
                for w in ow[:-1]:
                    nop = fresh_nop(ins.engine)
                    nop.sync_info = mybir.SyncInfo(on_wait=[w], on_update=[])
                    insts.insert(i, nop)
                    i += 1
                ins.sync_info = mybir.SyncInfo(on_wait=[ow[-1]],
                                               on_update=upd)
            i += 1


T, B, D = 100, 256, 1024
HH = [1024, 768, 512, 256, 10]  # H[l-1] -> H[l] for layer l in 1..4
NCORES = 8
BC = B // NCORES  # 32 batch per core
TB = 10           # time block
NBLK = T // TB
RING = 2 * TB     # ring slots for cur/spike buffers
NMR = 8           # membrane-ring slots (allows 4-step batched spike ops)
SG = 4            # spike-op step grouping (aligned to absolute step)
BETA = 0.95


def _kch(l):  # contraction chunks for layer l (input feature chunks)
    return (HH[l - 1] + 127) // 128


def _mch(l):  # output feature chunks
    return (HH[l] + 127) // 128


def _mpart(l):  # partitions used by last output chunk
    r = HH[l] % 128
    return 128 if r == 0 else r


def build_nc(repeat=1):
    nc = bass.Bass(target_bir_lowering=False, trn_type="TRN2")

    x_d = {
        p: nc.dram_tensor(f"x_{p}", [D, T * BC], F32R,
                          kind="ExternalInput") for p in PLANES
    }
    w_d = {}
    b_d = {}
    for l in range(1, 5):
        for p in PLANES:
            w_d[l, p] = nc.dram_tensor(
                f"w{l}{p}", [_kch(l), 128, HH[l]], F32R,
                kind="ExternalInput"
            )
        b_d[l] = nc.dram_tensor(f"b{l}", [HH[l]], F32, kind="ExternalInput")
    out_d = nc.dram_tensor("out", [10, BC], F32, kind="ExternalOutput")

    NB = TB * BC
    NT = NBLK * repeat  # total blocks emitted

    with tile.TileContext(nc) as tc:
        from contextlib import ExitStack

        with ExitStack() as ctx:
            wpool = ctx.enter_context(tc.tile_pool(name="weights", bufs=1))
            xpool = ctx.enter_context(tc.tile_pool(name="xblk", bufs=2))
            spool = ctx.enter_context(tc.tile_pool(name="state", bufs=1))
            psum = ctx.enter_context(
                tc.tile_pool(name="psum", bufs=7, space="PSUM")
            )
            pdum = ctx.enter_context(
                tc.tile_pool(name="pdum", bufs=1, space="PSUM")
            )

            # ---- per-chunk x DMA (separate tiles => per-chunk deps) ----
            def dma_x(blk):
                tiles = {}
                src = blk % NBLK
                for k in range(_kch(1)):
                    for p in PLANES:
                        xt = xpool.tile([128, NB], F32R,
                                        name=f"xb{p}{k}", tag=f"xb{p}{k}")
                        nc.sync.dma_start(
                            xt,
                            x_d[p][k * 128:(k + 1) * 128,
                                   src * NB:(src + 1) * NB],
                        )
                        tiles[p, k] = xt
                return tiles

            # ---- persistent state (allocate first: fixed SBUF homes) ----
            m_t = {}    # membrane rings, k-major: [pp, mc * NMR * BC]
            tmp_t = {}
            s_t = {}    # spike rings, k-major: [pp, mc * RING * BC]
            c_t = {}    # cur rings, t-major: [pp, RING * Fl]
            for l in range(1, 5):
                mc = _mch(l)
                mp = _mpart(l)
                Fl = mc * BC
                pp = mp if mc == 1 else 128
                m_t[l] = spool.tile([pp, mc * NMR * BC], F32, name=f"mem{l}")
                tmp_t[l] = spool.tile([pp, Fl], F32, name=f"tmp{l}")
                c_t[l] = spool.tile([pp, RING * Fl], F32, name=f"cur{l}")
                nc.vector.memset(m_t[l], 0.0)
                if l < 4:  # layer-4 spikes feed nothing
                    s_t[l] = spool.tile([pp, mc * RING * BC], F32R,
                                        name=f"spk{l}")
                    nc.vector.memset(s_t[l].bitcast(F32), 0.0)

            # ---- weights + biases: per-128-chunk tiles ----
            # DMA issue order sets the FIFO order on the DMA engine (and
            # the serial ~625ns/DMA HWDGE descriptor-gen): x(blk0) and W1
            # interleaved per chunk so the first matmul waits only on
            # chunk k0, then biases, x(blk1), then W2..W4 (first needed
            # one tick later).
            w_sb = {}   # (l, plane, k) -> [128, HH[l]]
            b_sb = {}
            x_tiles = {0: {}}
            for k in range(_kch(1)):
                for p in PLANES:
                    xt = xpool.tile([128, NB], F32R,
                                    name=f"xb{p}{k}", tag=f"xb{p}{k}")
                    nc.sync.dma_start(
                        xt, x_d[p][k * 128:(k + 1) * 128, 0:NB]
                    )
                    x_tiles[0][p, k] = xt
                for p in PLANES:
                    w_sb[1, p, k] = wpool.tile([128, HH[1]], F32R,
                                               name=f"wsb1{p}{k}")
                    nc.sync.dma_start(w_sb[1, p, k], w_d[1, p][k])

            for l in range(1, 5):
                mp = _mpart(l)
                b_sb[l] = wpool.tile([128, _mch(l)], F32, name=f"bsb{l}")
                nc.sync.dma_start(
                    b_sb[l][:mp, :],
                    b_d[l].rearrange("(c q) -> q c", q=mp)
                    if _mch(l) > 1
                    else b_d[l][:].unsqueeze(-1),
                )

            x_tiles[1] = dma_x(1)

            def dma_w(l):
                for k in range(_kch(l)):
                    for p in PLANES:
                        w_sb[l, p, k] = wpool.tile([128, HH[l]], F32R,
                                                   name=f"wsb{l}{p}{k}")
                        nc.sync.dma_start(w_sb[l, p, k], w_d[l, p][k])

            # W2 is needed one tick in; W3/W4 are deferred behind the
            # x(blk2) prefetch so they don't delay it in the DMA FIFO.
            dma_w(2)

            def lif_steps(l, b, split=False):
                """Sequential LIF updates for layer l over global block b.

                Two DVE ops per step (see module docstring); spike
                materialization on GPSIMD off the chain (skipped for l=4).

                split=True (used for the final block, where the chain is
                the critical path): run the recurrence as independent
                sub-chains over m-chunk pairs.  Each sub-chain only waits
                for its own chunks' cur drains, so it overlaps the tail
                of the same layer's matmul phase.  The recurrence is
                elementwise per neuron, so values are bit-identical.
                """
                mc = _mch(l)
                sb = (b % 2) * TB
                mr = m_t[l].rearrange("q (k n b) -> q k n b", n=NMR, b=BC)
                tmp3 = tmp_t[l].rearrange("q (k b) -> q k b", b=BC)
                c4 = c_t[l].rearrange("q (r k b) -> q r k b", r=RING, b=BC)
                if l < 4:
                    s4 = s_t[l].rearrange("q (k r b) -> q k r b",
                                          r=RING, b=BC)
                if not split:
                    # lanes: one full-width chain
                    pairs = [[(0, mc, 0, BC)]]
                elif mc >= 2:
                    # exactly two chunk-half lanes, ops interleaved: each
                    # lane's ~95ns dependent-op pipeline lag is covered by
                    # the other, at minimal extra per-op fixed cost
                    h = (mc + 1) // 2
                    pairs = [[(0, h, 0, BC), (h, mc, 0, BC)]]
                else:
                    # single chunk: interleave two batch-half lanes
                    h = BC // 2
                    pairs = [[(0, 1, 0, h), (0, 1, h, BC)]]
                for lanes in pairs:
                    t0 = 0  # start of the current spike group
                    for t in range(TB):
                        g = b * TB + t
                        cu, pv = g % NMR, (g - 1) % NMR
                        for k0, k1, b0, b1 in lanes:
                            # tmp = (mm * beta) - cur
                            nc.vector.scalar_tensor_tensor(
                                tmp3[:, k0:k1, b0:b1],
                                mr[:, k0:k1, pv, b0:b1], BETA,
                                c4[:, sb + t, k0:k1, b0:b1],
                                op0=ALU.mult, op1=ALU.subtract,
                            )
                        for k0, k1, b0, b1 in lanes:
                            # mm = (mm_prev is_lt -1) + tmp
                            nc.vector.scalar_tensor_tensor(
                                mr[:, k0:k1, cu, b0:b1],
                                mr[:, k0:k1, pv, b0:b1], -1.0,
                                tmp3[:, k0:k1, b0:b1],
                                op0=ALU.is_lt, op1=ALU.add,
                            )
                        # batched spikes: s[t0..t] = mm_ring < -1, flushed
                        # on SG-aligned absolute-step boundaries so ring
                        # slots stay contiguous (never wrap mod NMR).
                        # Tail chains flush every 2 steps so the next
                        # layer's matmuls unblock sooner.
                        sg = 2 if split else SG
                        if l < 4 and (g % sg == sg - 1 or t == TB - 1):
                            s0 = (b * TB + t0) % NMR
                            ng = t - t0 + 1
                            for k0, k1, b0, b1 in lanes:
                                nc.gpsimd.tensor_scalar(
                                    s4[:, k0:k1, sb + t0:sb + t + 1, b0:b1],
                                    mr[:, k0:k1, s0:s0 + ng, b0:b1], -1.0,
                                    None, op0=ALU.is_lt,
                                )
                            t0 = t + 1

            def layer_matmul(l, b, terms_of_k, k_outer=False,
                             splits=None):
                """Batched matmuls for layer l over global block b.

                terms_of_k(k) -> list of (plane, rhs AP [128, TB*BC]) to
                accumulate.  Per-PSUM accumulation order is k ascending
                with terms_of_k's order within k, identical for every
                loop nesting and step split (per-element accumulation
                order never changes).  Drains psum to c_t[l] with bias
                fused.

                splits: list of (t0, t1) step sub-ranges, each its own
                PSUM group + drain; used for the final block's layer-4
                matmul so its LIF chain starts before the last spike
                groups land.
                """
                mc = _mch(l)
                kc = _kch(l)
                mp = _mpart(l)
                sb = (b % 2) * TB
                c4 = c_t[l].rearrange("q (r k b) -> q r k b", r=RING, b=BC)
                nterms = len(terms_of_k(0))
                ntot = kc * nterms
                if splits is None:
                    splits = [(0, TB)]

                def emit(ps, m, pp, k, wp, rhs, i, t0, t1):
                    lhsT = w_sb[l, wp, k][:, m * 128:m * 128 + pp]
                    nc.tensor.matmul(
                        ps, lhsT, rhs[:, t0 * BC:t1 * BC],
                        start=(i == 0), stop=(i == ntot - 1),
                    )

                def drain(ps, m, pp, t0, t1):
                    nc.scalar.activation(
                        c4[:pp, sb + t0:sb + t1, m, :],
                        ps.rearrange("q (t b) -> q t b", b=BC),
                        ACTF.Identity,
                        bias=b_sb[l][:pp, m:m + 1],
                    )

                if k_outer:
                    t0, t1 = splits[0]
                    tiles = []
                    for m in range(mc):
                        pp = mp if m == mc - 1 else 128
                        tiles.append(
                            psum.tile([pp, (t1 - t0) * BC], F32,
                                      name=f"ps{l}", tag="ps")
                        )
                    for k in range(kc):
                        terms = terms_of_k(k)
                        for m in range(mc):
                            pp = mp if m == mc - 1 else 128
                            for j, (wp, rhs) in enumerate(terms):
                                emit(tiles[m], m, pp, k, wp, rhs,
                                     k * nterms + j, t0, t1)
                    for m in range(mc):
                        pp = mp if m == mc - 1 else 128
                        drain(tiles[m], m, pp, t0, t1)
                else:
                    for m in range(mc):
                        pp = mp if m == mc - 1 else 128
                        for t0, t1 in splits:
                            ps = psum.tile([pp, (t1 - t0) * BC], F32,
                                           name=f"ps{l}", tag="ps")
                            for k in range(kc):
                                for j, (wp, rhs) in \
                                        enumerate(terms_of_k(k)):
                                    emit(ps, m, pp, k, wp, rhs,
                                         k * nterms + j, t0, t1)
                            drain(ps, m, pp, t0, t1)

            def spike_terms(l, b):
                sb = (b % 2) * TB
                sl = s_t[l - 1]

                def terms(k, sl=sl, sb=sb):
                    rhs = sl[:, k * RING * BC + sb * BC:
                             k * RING * BC + sb * BC + NB]
                    return [(p, rhs) for p in PLANES]

                return terms

            def warm_pe(l, b, n):
                """Dummy matmuls gated on the tail chain's early spike
                groups: each executes as its group lands, keeping the PE
                p-state ramp alive across the chain-wait gap before the
                next layer's real matmuls (idle >~3us drops the clock)."""
                sb = (b % 2) * TB
                sl = s_t[l]
                ps = pdum.tile([128, 64], F32, name="psd", tag="psd")
                for i in range(n):
                    rhs = sl[:, sb * BC + i * 2 * BC:
                             sb * BC + (i * 2 + 2) * BC]
                    nc.tensor.matmul(ps, w_sb[1, "h", 0][:, :128],
                                     rhs, start=True, stop=True)

            # Software pipeline: at tick t, layer l works on block t-(l-1);
            # the PE's matmuls for tick t depend only on LIF work emitted
            # at tick t-1, so the PE never waits on the DVE in steady
            # state.  Repeats (timing runs) just extend the tick range.
            nticks = NT + 4
            for tick in range(nticks):
                # prefetch next tick's x block (this tick's is resident)
                if 2 <= tick + 1 < NT:
                    x_tiles[tick + 1] = dma_x(tick + 1)
                if tick == 1:
                    dma_w(3)
                    dma_w(4)
                for l in (1, 2, 3, 4):
                    b = tick - (l - 1)
                    if not (0 <= b < NT):
                        continue
                    if l == 1:
                        xb = x_tiles[b]
                        layer_matmul(
                            1, b,
                            lambda k: [
                                ("h", xb["h", k]),
                                ("h", xb["l", k]),
                                ("l", xb["h", k]),
                            ],
                            # k-outer only while DMA-paced (weights still
                            # streaming in); m-outer afterwards so cur
                            # drains land incrementally for the LIF chain
                            k_outer=(b <= 1),
                        )
                    else:
                        layer_matmul(
                            l, b, spike_terms(l, b),
                            # final-block L4: two step sub-ranges so its
                            # chain starts before the last s3 spikes land
                            # (4 tiles -> the <256-row penalty is ~0.3us)
                            splits=([(0, 8), (8, TB)]
                                    if l == 4 and b == NT - 1 else None),
                        )
                    if b - 1 >= 0:
                        lif_steps(l, b - 1)
                if tick >= NT - 1:
                    # drain the tail of each layer's LIF chain; split
                    # sub-chains overlap the final matmul phases.  The
                    # warm-up dummies (emitted after the chain, so they
                    # wait on its early spike groups) bridge the PE-idle
                    # window until the next layer's real matmuls.
                    for l in (1, 2, 3, 4):
                        if tick - (l - 1) == NT - 1:
                            lif_steps(l, tick - l + 1,
                                      split=True)
                            if l in (2, 3):
                                warm_pe(l, NT - 1, 3)

            # final-step membrane of layer 4 (sign-flipped; host negates)
            gfin = NT * TB - 1
            mr4 = m_t[4].rearrange("q (k n b) -> q k n b", n=NMR, b=BC)
            nc.sync.dma_start(out_d[:, :], mr4[:, 0, gfin % NMR, :])

    _split_multi_waits(nc)
    return nc


_NC_CACHE = None


def _get_nc():
    global _NC_CACHE
    if _NC_CACHE is None:
        _NC_CACHE = build_nc()
    return _NC_CACHE


def _rne11(a):
    """Round fp32 mantissa to 11 bits (RNE) -- the f32r operand grid."""
    u = np.ascontiguousarray(a, np.float32).view(np.uint32).astype(np.uint64)
    zb = 12  # 23 - 11
    lsb = (u >> zb) & 1
    add = lsb + ((1 << (zb - 1)) - 1)
    r = ((u + add) >> zb) << zb
    return r.astype(np.uint32).view(np.float32)


def _split2_11(a):
    """fp32 -> two 11-bit-mantissa planes with h + l == a exactly."""
    a = np.asarray(a, np.float32)
    h = _rne11(a)
    l = (a - h).astype(np.float32)
    return h, l


def prep_inputs(x, W1, b1, W2, b2, W3, b3, W4, b4):
    """Full inputs -> per-core in_maps."""
    Ws = {1: W1, 2: W2, 3: W3, 4: W4}
    bs = {1: b1, 2: b2, 3: b3, 4: b4}
    shared = {}
    for l in range(1, 5):
        wt = np.ascontiguousarray(
            np.asarray(Ws[l], np.float32).T.reshape(_kch(l), 128, HH[l])
        )
        wh, wl = _split2_11(wt)
        shared[f"w{l}h"] = wh
        shared[f"w{l}l"] = wl
        shared[f"b{l}"] = np.ascontiguousarray(bs[l], dtype=np.float32)
    in_maps = []
    for c in range(NCORES):
        xc = np.asarray(x[:, c * BC:(c + 1) * BC, :], np.float32)
        xc = np.ascontiguousarray(xc.transpose(2, 0, 1).reshape(D, T * BC))
        xh, xl = _split2_11(xc)
        m = {"x_h": xh, "x_l": xl}
        m.update(shared)
        in_maps.append(m)
    return in_maps


def run(in_maps, trace=False):
    nc = _get_nc()
    return bass_utils.run_bass_kernel_spmd(
        nc, in_maps, core_ids=list(range(NCORES)), trace=trace
    )


def kernel(**inputs):
    in_maps = prep_inputs(**inputs)
    res = run(in_maps)
    out = np.empty((B, 10), dtype=np.float32)
    for c in range(NCORES):
        # device tracks -m (sign-flipped LIF chain); negate here
        out[c * BC:(c + 1) * BC, :] = -res.results[c]["out"].T
    return out


def bench(in_maps, iters=20, nc=None):
    """Repeat-execute the kernel via a cached sharded jit; returns list of
    per-call wall times (seconds).  Mirrors bass2jax.run_bass_via_pjrt's
    multi-core path but keeps inputs device-resident across calls."""
    import time

    import jax
    import concourse.mybir as mybir_
    from jax.sharding import Mesh, PartitionSpec
    from jax.experimental.shard_map import shard_map
    from concourse import bass2jax

    bass2jax.install_neuronx_cc_hook()
    if nc is None:
        nc = _get_nc()

    part_name = (nc.partition_id_tensor.name
                 if nc.partition_id_tensor else None)
    in_names, out_names, out_avals, zero_outs = [], [], [], []
    for alloc in nc.m.functions[0].allocations:
        if not isinstance(alloc, mybir_.MemoryLocationSet):
            continue
        name = alloc.memorylocations[0].name
        if alloc.kind == "ExternalInput":
            if name != part_name:
                in_names.append(name)
        elif alloc.kind == "ExternalOutput":
            out_names.append(name)
            shape = tuple(alloc.tensor_shape)
            dtype = mybir_.dt.np(alloc.dtype)
            out_avals.append(jax.core.ShapedArray(shape, dtype))
            zero_outs.append(np.zeros(shape, dtype))
    n_params = len(in_names)
    all_in_names = in_names + out_names
    if part_name is not None:
        all_in_names = all_in_names + [part_name]

    def _body(*args):
        operands = list(args)
        if part_name is not None:
            operands.append(bass2jax.partition_id_tensor())
        outs = bass2jax._bass_exec_p.bind(
            *operands,
            out_avals=tuple(out_avals),
            in_names=tuple(all_in_names),
            out_names=tuple(out_names),
            lowering_input_output_aliases=(),
            sim_require_finite=True,
            sim_require_nnan=True,
            nc=nc,
        )
        return tuple(outs)

    devices = jax.devices()[:NCORES]
    mesh = Mesh(np.asarray(devices), ("core",))
    n_outs = len(out_names)
    sharded = jax.jit(
        shard_map(
            _body, mesh=mesh,
            in_specs=(PartitionSpec("core"),) * (n_params + n_outs),
            out_specs=(PartitionSpec("core"),) * n_outs,
            check_rep=False,
        ),
        donate_argnums=tuple(range(n_params, n_params + n_outs)),
        keep_unused=True,
    )
    concat_in = [
        np.concatenate([np.asarray(m[nm]) for m in in_maps], axis=0)
        for nm in in_names
    ]
    concat_in = jax.device_put(concat_in)
    zeros = [
        np.zeros((NCORES * z.shape[0], *z.shape[1:]), z.dtype)
        for z in zero_outs
    ]
    # warmup (compile)
    out = sharded(*concat_in, *zeros)
    jax.block_until_ready(out)
    times = []
    for _ in range(iters):
        t0 = time.perf_counter()
        out = sharded(*concat_in, *zeros)
        jax.block_until_ready(out)
        times.append(time.perf_counter() - t0)
    return times


# revision 29
# speedup vs baseline: 1.0067x; 1.0002x over previous
"""Trainium2 Bass kernel for EnhancedSpikingAudioNet (4-layer LIF SNN).

Network (eval mode): for t in 0..99:
    s1,m1 = LIF(x_t @ W1.T + b1, m1)
    s2,m2 = LIF(s1 @ W2.T + b2, m2)
    s3,m3 = LIF(s2 @ W3.T + b3, m3)
    s4,m4 = LIF(s3 @ W4.T + b4, m4)
returns m4 (final step), shape [B=256, 10].

LIF (snnTorch Leaky, reset_mechanism='subtract', beta=.95, thr=1):
    reset = (m_prev > 1);  m = beta*m_prev + cur - reset;  s = (m > 1)

Strategy: data-parallel over batch (32 per core, 8 cores).  Inside a
core, time is blocked (TB=10): all matmuls for a block are batched over
the block's 10 steps (moving free dim N=320); only the per-step LIF
update is sequential.  Layout: features on partitions (128-chunks),
(t, batch) on the free dim.  PSUM drains to SBUF via ScalarE with the
layer bias fused in.

LIF chain (bit-identical sign-flipped form): track mm = -m.  Per step,
two same-engine DVE ops (shortest possible serial chain):
    tmp   = (mm * beta) - cur            # == -(beta*m + cur)
    mm_t  = (mm is_lt -1) + tmp          # == -((beta*m+cur) - (m>1))
Every fp32 op is the exact sign-mirror of the upstream form (RNE is
sign-symmetric), so results match the previous 3-op chain bit-for-bit;
the host negates the final mm4.  Spikes for the next layer's matmul,
s = (mm is_lt -1), are generated per step on GPSIMD off the critical
path (layer 4's spikes feed nothing and are skipped).

Numerics: the spike cascade amplifies matmul noise (a plain f32r
matmul gives ~16% output error; even exact-fp32 summation-order noise
gives ~1.6%; gate is 2%), so matmuls must be fp32-faithful.  Hardware
probing established: float32r = operands rounded RNE to 11 mantissa
bits, then EXACT products with clean fp32 accumulation, at full PE
rate (1 cyc/row) for moving dims >= 256.  Every fp32 tensor is split
host-side into two 11-bit planes (h = rne11(a), l = a-h; both planes
survive the hw operand rounding unchanged).  Layer 1 accumulates
wh@xh + wh@xl + wl@xh (dropping wl@xl ~ 2^-24), layers 2-4 accumulate
wh@s + wl@s.  Accumulation order per PSUM bank (k ascending, fixed
term order) is kept identical across scheduling changes.

Schedule: weights are loaded per-128-chunk and layer 1 runs its k loop
OUTERMOST (6 PSUM banks open at once), so the first matmul needs only
x(blk0) chunk k0 + W1 chunk k0 instead of all of W1; DMA issue order
(x blk0 chunks, W1 chunks h/l-interleaved, biases, x blk1 chunks,
W2..W4) keeps the PE fed from ~6us on a FIFO DMA engine.
"""

import os
import sys

import numpy as np

for _p in ("/opt/trn_rl_repo",):
    if os.path.isdir(_p) and _p not in sys.path:
        sys.path.insert(0, _p)

import concourse.bass as bass
import concourse.mybir as mybir
import concourse.tile as tile
from concourse import bass_utils

F32 = mybir.dt.float32
F32R = mybir.dt.float32r
ALU = mybir.AluOpType
ACTF = mybir.ActivationFunctionType
PLANES = ("h", "l")  # 11-bit f32r planes


def _patch_tail_drain():
    """This container's walrus allows only ONE sync-wait on a Drain
    instruction; Tile's kernel-tail drain can carry several (one per DMA
    HW queue).  Spread the waits across consecutive drains instead."""
    from concourse.vector_clock import ScopedClock

    if getattr(tile.TileContext, "_tail_drain_patched", False):
        return

    def _drain_and_barrier(self, tick_clock, wait_clock):
        drain_inst = self.nc.sync.drain()
        wait_clock.add_sem_waits(
            drain_inst.ins, ScopedClock({None: tick_clock.global_clock})
        )
        si = drain_inst.ins.sync_info
        if si is not None and si.on_wait and len(si.on_wait) > 1:
            waits = list(si.on_wait)
            drain_inst.ins.sync_info = mybir.SyncInfo(
                on_wait=[waits[0]], on_update=list(si.on_update or [])
            )
            for w in waits[1:]:
                extra = self.nc.sync.drain()
                extra.ins.sync_info = mybir.SyncInfo(on_wait=[w], on_update=[])

        self.nc.all_engine_barrier()
        assert self.sems is not None
        popped = self.nc._tile_sem_poison_stack.pop()
        assert popped is self._sem_poison
        self.nc.clear_and_free_semaphores(
            list(self.sems.allocated().values())
        )
        self.nc.all_engine_barrier()

    tile.TileContext._drain_and_barrier = _drain_and_barrier
    tile.TileContext._tail_drain_patched = True


_patch_tail_drain()


def _split_multi_waits(nc):
    """This walrus build rejects instructions carrying more than one
    sync-wait (a DMA-HW-queue sem wait expands into several wait
    commands).  Give every instruction at most one wait; extras go onto
    same-engine NOPs inserted immediately before it."""

    def fresh_nop(engine):
        eng = nc.engines[engine]
        bi = eng.nop(nofuse=True)
        raw = bi.ins
        # nop() appended raw to the current bb -- remove it, we re-insert.
        for bb in nc.main_func.blocks:
            try:
                bb.instructions.remove(raw)
                break
            except ValueError:
                continue
        return raw

    for bb in nc.main_func.blocks:
        insts = bb.instructions
        i = 0
        while i < len(insts):
            ins = insts[i]
            si = getattr(ins, "sync_info", None)
            ow = list(si.on_wait) if (si is not None and si.on_wait) else []
            if len(ow) > 1:
                upd = list(si.on_update or [])
                for w in ow[:-1]:
                    nop = fresh_nop(ins.engine)
                    nop.sync_info = mybir.SyncInfo(on_wait=[w], on_update=[])
                    insts.insert(i, nop)
                    i += 1
                ins.sync_info = mybir.SyncInfo(on_wait=[ow[-1]],
                                               on_update=upd)
            i += 1


T, B, D = 100, 256, 1024
HH = [1024, 768, 512, 256, 10]  # H[l-1] -> H[l] for layer l in 1..4
NCORES = 8
BC = B // NCORES  # 32 batch per core
TB = 10           # time block
NBLK = T // TB
RING = 2 * TB     # ring slots for cur/spike buffers
NMR = 8           # membrane-ring slots (allows 4-step batched spike ops)
SG = 4            # spike-op step grouping (aligned to absolute step)
BETA = 0.95


def _kch(l):  # contraction chunks for layer l (input feature chunks)
    return (HH[l - 1] + 127) // 128


def _mch(l):  # output feature chunks
    return (HH[l] + 127) // 128


def _mpart(l):  # partitions used by last output chunk
    r = HH[l] % 128
    return 128 if r == 0 else r


def build_nc(repeat=1):
    nc = bass.Bass(target_bir_lowering=False, trn_type="TRN2")

    x_d = {
        p: nc.dram_tensor(f"x_{p}", [D, T * BC], F32R,
                          kind="ExternalInput") for p in PLANES
    }
    w_d = {}
    b_d = {}
    for l in range(1, 5):
        for p in PLANES:
            w_d[l, p] = nc.dram_tensor(
                f"w{l}{p}", [_kch(l), 128, HH[l]], F32R,
                kind="ExternalInput"
            )
        b_d[l] = nc.dram_tensor(f"b{l}", [HH[l]], F32, kind="ExternalInput")
    out_d = nc.dram_tensor("out", [10, BC], F32, kind="ExternalOutput")

    NB = TB * BC
    NT = NBLK * repeat  # total blocks emitted

    with tile.TileContext(nc) as tc:
        from contextlib import ExitStack

        with ExitStack() as ctx:
            wpool = ctx.enter_context(tc.tile_pool(name="weights", bufs=1))
            xpool = ctx.enter_context(tc.tile_pool(name="xblk", bufs=2))
            spool = ctx.enter_context(tc.tile_pool(name="state", bufs=1))
            psum = ctx.enter_context(
                tc.tile_pool(name="psum", bufs=7, space="PSUM")
            )
            pdum = ctx.enter_context(
                tc.tile_pool(name="pdum", bufs=1, space="PSUM")
            )

            # ---- x DMA: one tile per plane; per-chunk slice DMAs while
            # the startup is DMA-paced (Tile deps are slice-granular), a
            # single whole-plane DMA per tick in steady state ----
            KC1 = _kch(1)

            def dma_x(blk, chunked=False):
                tiles = {}
                src = blk % NBLK
                for p in PLANES:
                    xt = xpool.tile([128, KC1 * NB], F32R,
                                    name=f"xb{p}", tag=f"xb{p}")
                    tiles[p] = xt
                if chunked:
                    for k in range(KC1):
                        for p in PLANES:
                            nc.sync.dma_start(
                                tiles[p][:, k * NB:(k + 1) * NB],
                                x_d[p][k * 128:(k + 1) * 128,
                                       src * NB:(src + 1) * NB],
                            )
                else:
                    for p in PLANES:
                        nc.sync.dma_start(
                            tiles[p].rearrange("q (k n) -> q k n", n=NB),
                            x_d[p][:, src * NB:(src + 1) * NB].rearrange(
                                "(k q) n -> q k n", q=128
                            ),
                        )
                return tiles

            # ---- persistent state (allocate first: fixed SBUF homes) ----
            m_t = {}    # membrane rings, k-major: [pp, mc * NMR * BC]
            tmp_t = {}
            s_t = {}    # spike rings, k-major: [pp, mc * RING * BC]
            c_t = {}    # cur rings, t-major: [pp, RING * Fl]
            for l in range(1, 5):
                mc = _mch(l)
                mp = _mpart(l)
                Fl = mc * BC
                pp = mp if mc == 1 else 128
                m_t[l] = spool.tile([pp, mc * NMR * BC], F32, name=f"mem{l}")
                tmp_t[l] = spool.tile([pp, Fl], F32, name=f"tmp{l}")
                c_t[l] = spool.tile([pp, RING * Fl], F32, name=f"cur{l}")
                nc.vector.memset(m_t[l], 0.0)
                if l < 4:  # layer-4 spikes feed nothing
                    s_t[l] = spool.tile([pp, mc * RING * BC], F32R,
                                        name=f"spk{l}")
                    nc.vector.memset(s_t[l].bitcast(F32), 0.0)

            # ---- weights + biases: per-128-chunk tiles ----
            # DMA issue order sets the FIFO order on the DMA engine (and
            # the serial ~625ns/DMA HWDGE descriptor-gen): x(blk0) and W1
            # interleaved per chunk so the first matmul waits only on
            # chunk k0, then biases, x(blk1), then W2..W4 (first needed
            # one tick later).
            w_sb = {}   # (l, plane, k) -> [128, HH[l]]
            b_sb = {}
            x_tiles = {0: {p: xpool.tile([128, KC1 * NB], F32R,
                                         name=f"xb{p}", tag=f"xb{p}")
                          for p in PLANES}}
            for k in range(KC1):
                for p in PLANES:
                    nc.sync.dma_start(
                        x_tiles[0][p][:, k * NB:(k + 1) * NB],
                        x_d[p][k * 128:(k + 1) * 128, 0:NB],
                    )
                for p in PLANES:
                    w_sb[1, p, k] = wpool.tile([128, HH[1]], F32R,
                                               name=f"wsb1{p}{k}")
                    nc.sync.dma_start(w_sb[1, p, k], w_d[1, p][k])

            for l in range(1, 5):
                mp = _mpart(l)
                b_sb[l] = wpool.tile([128, _mch(l)], F32, name=f"bsb{l}")
                nc.sync.dma_start(
                    b_sb[l][:mp, :],
                    b_d[l].rearrange("(c q) -> q c", q=mp)
                    if _mch(l) > 1
                    else b_d[l][:].unsqueeze(-1),
                )

            x_tiles[1] = dma_x(1, chunked=True)

            def dma_w(l):
                for k in range(_kch(l)):
                    for p in PLANES:
                        w_sb[l, p, k] = wpool.tile([128, HH[l]], F32R,
                                                   name=f"wsb{l}{p}{k}")
                        nc.sync.dma_start(w_sb[l, p, k], w_d[l, p][k])

            # W2 is needed one tick in; W3/W4 are deferred behind the
            # x(blk2) prefetch so they don't delay it in the DMA FIFO.
            dma_w(2)

            def lif_steps(l, b, split=False):
                """Sequential LIF updates for layer l over global block b.

                Two DVE ops per step (see module docstring); spike
                materialization on GPSIMD off the chain (skipped for l=4).

                split=True (used for the final block, where the chain is
                the critical path): run the recurrence as independent
                sub-chains over m-chunk pairs.  Each sub-chain only waits
                for its own chunks' cur drains, so it overlaps the tail
                of the same layer's matmul phase.  The recurrence is
                elementwise per neuron, so values are bit-identical.
                """
                mc = _mch(l)
                sb = (b % 2) * TB
                mr = m_t[l].rearrange("q (k n b) -> q k n b", n=NMR, b=BC)
                tmp3 = tmp_t[l].rearrange("q (k b) -> q k b", b=BC)
                c4 = c_t[l].rearrange("q (r k b) -> q r k b", r=RING, b=BC)
                if l < 4:
                    s4 = s_t[l].rearrange("q (k r b) -> q k r b",
                                          r=RING, b=BC)
                if not split:
                    # lanes: one full-width chain
                    pairs = [[(0, mc, 0, BC)]]
                elif mc >= 2:
                    # exactly two chunk-half lanes, ops interleaved: each
                    # lane's ~95ns dependent-op pipeline lag is covered by
                    # the other, at minimal extra per-op fixed cost
                    h = (mc + 1) // 2
                    pairs = [[(0, h, 0, BC), (h, mc, 0, BC)]]
                else:
                    # single chunk: interleave two batch-half lanes
                    h = BC // 2
                    pairs = [[(0, 1, 0, h), (0, 1, h, BC)]]
                for lanes in pairs:
                    t0 = 0  # start of the current spike group
                    for t in range(TB):
                        g = b * TB + t
                        cu, pv = g % NMR, (g - 1) % NMR
                        for k0, k1, b0, b1 in lanes:
                            # tmp = (mm * beta) - cur
                            nc.vector.scalar_tensor_tensor(
                                tmp3[:, k0:k1, b0:b1],
                                mr[:, k0:k1, pv, b0:b1], BETA,
                                c4[:, sb + t, k0:k1, b0:b1],
                                op0=ALU.mult, op1=ALU.subtract,
                            )
                        for k0, k1, b0, b1 in lanes:
                            # mm = (mm_prev is_lt -1) + tmp
                            nc.vector.scalar_tensor_tensor(
                                mr[:, k0:k1, cu, b0:b1],
                                mr[:, k0:k1, pv, b0:b1], -1.0,
                                tmp3[:, k0:k1, b0:b1],
                                op0=ALU.is_lt, op1=ALU.add,
                            )
                        # batched spikes: s[t0..t] = mm_ring < -1, flushed
                        # on SG-aligned absolute-step boundaries so ring
                        # slots stay contiguous (never wrap mod NMR).
                        # Tail chains flush every 2 steps so the next
                        # layer's matmuls unblock sooner.
                        sg = 2 if split else SG
                        if l < 4 and (g % sg == sg - 1 or t == TB - 1):
                            s0 = (b * TB + t0) % NMR
                            ng = t - t0 + 1
                            for k0, k1, b0, b1 in lanes:
                                nc.gpsimd.tensor_scalar(
                                    s4[:, k0:k1, sb + t0:sb + t + 1, b0:b1],
                                    mr[:, k0:k1, s0:s0 + ng, b0:b1], -1.0,
                                    None, op0=ALU.is_lt,
                                )
                            t0 = t + 1

            def layer_matmul(l, b, terms_of_k, k_outer=False,
                             splits=None):
                """Batched matmuls for layer l over global block b.

                terms_of_k(k) -> list of (plane, rhs AP [128, TB*BC]) to
                accumulate.  Per-PSUM accumulation order is k ascending
                with terms_of_k's order within k, identical for every
                loop nesting and step split (per-element accumulation
                order never changes).  Drains psum to c_t[l] with bias
                fused.

                splits: list of (t0, t1) step sub-ranges, each its own
                PSUM group + drain; used for the final block's layer-4
                matmul so its LIF chain starts before the last spike
                groups land.
                """
                mc = _mch(l)
                kc = _kch(l)
                mp = _mpart(l)
                sb = (b % 2) * TB
                c4 = c_t[l].rearrange("q (r k b) -> q r k b", r=RING, b=BC)
                nterms = len(terms_of_k(0))
                ntot = kc * nterms
                if splits is None:
                    splits = [(0, TB)]

                def emit(ps, m, pp, k, wp, rhs, i, t0, t1):
                    lhsT = w_sb[l, wp, k][:, m * 128:m * 128 + pp]
                    nc.tensor.matmul(
                        ps, lhsT, rhs[:, t0 * BC:t1 * BC],
                        start=(i == 0), stop=(i == ntot - 1),
                    )

                def drain(ps, m, pp, t0, t1):
                    nc.scalar.activation(
                        c4[:pp, sb + t0:sb + t1, m, :],
                        ps.rearrange("q (t b) -> q t b", b=BC),
                        ACTF.Identity,
                        bias=b_sb[l][:pp, m:m + 1],
                    )

                if k_outer:
                    t0, t1 = splits[0]
                    tiles = []
                    for m in range(mc):
                        pp = mp if m == mc - 1 else 128
                        tiles.append(
                            psum.tile([pp, (t1 - t0) * BC], F32,
                                      name=f"ps{l}", tag="ps")
                        )
                    for k in range(kc):
                        terms = terms_of_k(k)
                        for m in range(mc):
                            pp = mp if m == mc - 1 else 128
                            for j, (wp, rhs) in enumerate(terms):
                                emit(tiles[m], m, pp, k, wp, rhs,
                                     k * nterms + j, t0, t1)
                    for m in range(mc):
                        pp = mp if m == mc - 1 else 128
                        drain(tiles[m], m, pp, t0, t1)
                else:
                    for m in range(mc):
                        pp = mp if m == mc - 1 else 128
                        for t0, t1 in splits:
                            ps = psum.tile([pp, (t1 - t0) * BC], F32,
                                           name=f"ps{l}", tag="ps")
                            for k in range(kc):
                                for j, (wp, rhs) in \
                                        enumerate(terms_of_k(k)):
                                    emit(ps, m, pp, k, wp, rhs,
                                         k * nterms + j, t0, t1)
                            drain(ps, m, pp, t0, t1)

            def spike_terms(l, b):
                sb = (b % 2) * TB
                sl = s_t[l - 1]

                def terms(k, sl=sl, sb=sb):
                    rhs = sl[:, k * RING * BC + sb * BC:
                             k * RING * BC + sb * BC + NB]
                    return [(p, rhs) for p in PLANES]

                return terms

            def warm_pe(l, b, n):
                """Dummy matmuls gated on the tail chain's early spike
                groups: each executes as its group lands, keeping the PE
                p-state ramp alive across the chain-wait gap before the
                next layer's real matmuls (idle >~3us drops the clock)."""
                sb = (b % 2) * TB
                sl = s_t[l]
                ps = pdum.tile([128, 64], F32, name="psd", tag="psd")
                for i in range(n):
                    rhs = sl[:, sb * BC + i * 2 * BC:
                             sb * BC + (i * 2 + 2) * BC]
                    nc.tensor.matmul(ps, w_sb[1, "h", 0][:, :128],
                                     rhs, start=True, stop=True)

            # Software pipeline: at tick t, layer l works on block t-(l-1);
            # the PE's matmuls for tick t depend only on LIF work emitted
            # at tick t-1, so the PE never waits on the DVE in steady
            # state.  Repeats (timing runs) just extend the tick range.
            nticks = NT + 4
            for tick in range(nticks):
                # prefetch next tick's x block (this tick's is resident)
                if 2 <= tick + 1 < NT:
                    x_tiles[tick + 1] = dma_x(tick + 1)
                if tick == 1:
                    dma_w(3)
                    dma_w(4)
                for l in (1, 2, 3, 4):
                    b = tick - (l - 1)
                    if not (0 <= b < NT):
                        continue
                    if l == 1:
                        xb = x_tiles[b]
                        layer_matmul(
                            1, b,
                            lambda k: [
                                ("h", xb["h"][:, k * NB:(k + 1) * NB]),
                                ("h", xb["l"][:, k * NB:(k + 1) * NB]),
                                ("l", xb["h"][:, k * NB:(k + 1) * NB]),
                            ],
                            # k-outer only while DMA-paced (weights still
                            # streaming in); m-outer afterwards so cur
                            # drains land incrementally for the LIF chain
                            k_outer=(b <= 1),
                        )
                    else:
                        layer_matmul(
                            l, b, spike_terms(l, b),
                            # final-block L4: two step sub-ranges so its
                            # chain starts before the last s3 spikes land
                            # (4 tiles -> the <256-row penalty is ~0.3us)
                            splits=([(0, 8), (8, TB)]
                                    if l == 4 and b == NT - 1 else None),
                        )
                    if b - 1 >= 0:
                        lif_steps(l, b - 1)
                if tick >= NT - 1:
                    # drain the tail of each layer's LIF chain; split
                    # sub-chains overlap the final matmul phases.  The
                    # warm-up dummies (emitted after the chain, so they
                    # wait on its early spike groups) bridge the PE-idle
                    # window until the next layer's real matmuls.
                    for l in (1, 2, 3, 4):
                        if tick - (l - 1) == NT - 1:
                            lif_steps(l, tick - l + 1,
                                      split=True)
                            if l in (2, 3):
                                warm_pe(l, NT - 1, 3)

            # final-step membrane of layer 4 (sign-flipped; host negates)
            gfin = NT * TB - 1
            mr4 = m_t[4].rearrange("q (k n b) -> q k n b", n=NMR, b=BC)
            nc.sync.dma_start(out_d[:, :], mr4[:, 0, gfin % NMR, :])

    _split_multi_waits(nc)
    return nc


_NC_CACHE = None


def _get_nc():
    global _NC_CACHE
    if _NC_CACHE is None:
        _NC_CACHE = build_nc()
    return _NC_CACHE


def _rne11(a):
    """Round fp32 mantissa to 11 bits (RNE) -- the f32r operand grid."""
    u = np.ascontiguousarray(a, np.float32).view(np.uint32).astype(np.uint64)
    zb = 12  # 23 - 11
    lsb = (u >> zb) & 1
    add = lsb + ((1 << (zb - 1)) - 1)
    r = ((u + add) >> zb) << zb
    return r.astype(np.uint32).view(np.float32)


def _split2_11(a):
    """fp32 -> two 11-bit-mantissa planes with h + l == a exactly."""
    a = np.asarray(a, np.float32)
    h = _rne11(a)
    l = (a - h).astype(np.float32)
    return h, l


def prep_inputs(x, W1, b1, W2, b2, W3, b3, W4, b4):
    """Full inputs -> per-core in_maps."""
    Ws = {1: W1, 2: W2, 3: W3, 4: W4}
    bs = {1: b1, 2: b2, 3: b3, 4: b4}
    shared = {}
    for l in range(1, 5):
        wt = np.ascontiguousarray(
            np.asarray(Ws[l], np.float32).T.reshape(_kch(l), 128, HH[l])
        )
        wh, wl = _split2_11(wt)
        shared[f"w{l}h"] = wh
        shared[f"w{l}l"] = wl
        shared[f"b{l}"] = np.ascontiguousarray(bs[l], dtype=np.float32)
    in_maps = []
    for c in range(NCORES):
        xc = np.asarray(x[:, c * BC:(c + 1) * BC, :], np.float32)
        xc = np.ascontiguousarray(xc.transpose(2, 0, 1).reshape(D, T * BC))
        xh, xl = _split2_11(xc)
        m = {"x_h": xh, "x_l": xl}
        m.update(shared)
        in_maps.append(m)
    return in_maps


def run(in_maps, trace=False):
    nc = _get_nc()
    return bass_utils.run_bass_kernel_spmd(
        nc, in_maps, core_ids=list(range(NCORES)), trace=trace
    )


def kernel(**inputs):
    in_maps = prep_inputs(**inputs)
    res = run(in_maps)
    out = np.empty((B, 10), dtype=np.float32)
    for c in range(NCORES):
        # device tracks -m (sign-flipped LIF chain); negate here
        out[c * BC:(c + 1) * BC, :] = -res.results[c]["out"].T
    return out


def bench(in_maps, iters=20, nc=None):
    """Repeat-execute the kernel via a cached sharded jit; returns list of
    per-call wall times (seconds).  Mirrors bass2jax.run_bass_via_pjrt's
    multi-core path but keeps inputs device-resident across calls."""
    import time

    import jax
    import concourse.mybir as mybir_
    from jax.sharding import Mesh, PartitionSpec
    from jax.experimental.shard_map import shard_map
    from concourse import bass2jax

    bass2jax.install_neuronx_cc_hook()
    if nc is None:
        nc = _get_nc()

    part_name = (nc.partition_id_tensor.name
                 if nc.partition_id_tensor else None)
    in_names, out_names, out_avals, zero_outs = [], [], [], []
    for alloc in nc.m.functions[0].allocations:
        if not isinstance(alloc, mybir_.MemoryLocationSet):
            continue
        name = alloc.memorylocations[0].name
        if alloc.kind == "ExternalInput":
            if name != part_name:
                in_names.append(name)
        elif alloc.kind == "ExternalOutput":
            out_names.append(name)
            shape = tuple(alloc.tensor_shape)
            dtype = mybir_.dt.np(alloc.dtype)
            out_avals.append(jax.core.ShapedArray(shape, dtype))
            zero_outs.append(np.zeros(shape, dtype))
    n_params = len(in_names)
    all_in_names = in_names + out_names
    if part_name is not None:
        all_in_names = all_in_names + [part_name]

    def _body(*args):
        operands = list(args)
        if part_name is not None:
            operands.append(bass2jax.partition_id_tensor())
        outs = bass2jax._bass_exec_p.bind(
            *operands,
            out_avals=tuple(out_avals),
            in_names=tuple(all_in_names),
            out_names=tuple(out_names),
            lowering_input_output_aliases=(),
            sim_require_finite=True,
            sim_require_nnan=True,
            nc=nc,
        )
        return tuple(outs)

    devices = jax.devices()[:NCORES]
    mesh = Mesh(np.asarray(devices), ("core",))
    n_outs = len(out_names)
    sharded = jax.jit(
        shard_map(
            _body, mesh=mesh,
            in_specs=(PartitionSpec("core"),) * (n_params + n_outs),
            out_specs=(PartitionSpec("core"),) * n_outs,
            check_rep=False,
        ),
        donate_argnums=tuple(range(n_params, n_params + n_outs)),
        keep_unused=True,
    )
    concat_in = [
        np.concatenate([np.asarray(m[nm]) for m in in_maps], axis=0)
        for nm in in_names
    ]
    concat_in = jax.device_put(concat_in)
    zeros = [
        np.zeros((NCORES * z.shape[0], *z.shape[1:]), z.dtype)
        for z in zero_outs
    ]
    # warmup (compile)
    out = sharded(*concat_in, *zeros)
    jax.block_until_ready(out)
    times = []
    for _ in range(iters):
        t0 = time.perf_counter()
        out = sharded(*concat_in, *zeros)
        jax.block_until_ready(out)
        times.append(time.perf_counter() - t0)
    return times
